# revision 3
# baseline (speedup 1.0000x reference)
"""Trainium2 Bass kernel for Luong local-p sparse attention (v2).

Math (per batch n, full shapes N=64, L=258, H=1024, Q=256):
    score = (h_t @ W_a) @ enc^T           masked to window [p_t-16, p_t+16]
    align = softmax(score) * gauss(p_t)
    out   = tanh([align @ enc, h_t] @ W_c^T)

Only a 32-wide slice of enc can survive the mask for non-integer p_t, so the
kernel gathers 32-wide windows host-side (W=32 -> 4 windows pack exactly into
128 PE partitions).  Score path (u = W_a-transform of windows, score, softmax)
runs in fp32r for softmax accuracy; the heavy W_c matmuls (dec @ W_c2T, window
@ W_c1T, align @ v) run in bf16 (same PE rate, half the DMA/SBUF).

    u  = W_a-transform of window   (uT[h, (n,j)]  = sum_k W_aT[k,h] enc_w[(n,j),k])
    s  = uT^T-partial scores       (score^T[j, q] = sum_h uT[h,j] h_t[q,h])
    softmax over j (32 rows) j-major with a 4th-power renormalization trick
    v  = W_c1-transform of window  (bf16), gaussian folded on PSUM evacuation
    out = tanh(dec_b16 @ W_c2T_b16 + align_b16^T.T @ v)

Schedule: PE warm-up matmuls cover the initial DMA window (keeps the HAM
clock gate at 8/8); u phase streams kc-major behind the enc/W_aT DMA; v and
batch 0 overlap the W_c1/W_c2/dec streams; batches pipeline as in the
baseline with dec prefetched 2 batches ahead.  Data parallel over batch:
8 batches per core x 8 cores.
"""

import numpy as np
import ml_dtypes

import concourse.bass as bass
import concourse.bacc as bacc
import concourse.mybir as mybir
import concourse.tile as tile
from concourse.bass_utils import run_bass_kernel_spmd

# Problem constants (hardcoded per harness contract).
N, L, H, Q = 64, 258, 1024, 256
WINDOW = 16.0
DEV_POW = 128.0
NCORES = 8
B = N // NCORES  # batches per core
W = 32           # window width (max live positions for non-integer p_t)
HC = H // 128    # h-chunks of 128 (PE contraction tiles)
F32 = mybir.dt.float32
F32R = mybir.dt.float32r
BF16 = mybir.dt.bfloat16
AF = mybir.ActivationFunctionType

# exp is computed as t = exp(s/4 + bias); bias = LOG_ALPHA keeps the
# column-sum T = sum_j t below fp32 max.  alpha cancels in w = t/T.
LOG_ALPHA = -4.8520302  # -7*ln(2)
MASK_BIAS = -10000.0    # exp(<= -2500) == 0 in fp32
N_WARM = 140            # PE warm-up matmuls spanning the initial DMA wait


def build_nc() -> bass.Bass:
    nc = bacc.Bacc()
    enc_wT = nc.declare_dram_parameter("enc_wT", [H, B * W], F32R, isOutput=False)
    enc_wTb = nc.declare_dram_parameter("enc_wTb", [H, B * W], BF16, isOutput=False)
    dec_hT = nc.declare_dram_parameter("dec_hT", [H, B * Q], F32R, isOutput=False)
    W_aT = nc.declare_dram_parameter("W_aT", [H, H], F32R, isOutput=False)
    W_c1T = nc.declare_dram_parameter("W_c1T", [H, H], BF16, isOutput=False)
    W_c2T = nc.declare_dram_parameter("W_c2T", [H, H], BF16, isOutput=False)
    biasT = nc.declare_dram_parameter("biasT", [W, B], F32, isOutput=False)
    gPackT = nc.declare_dram_parameter("gPackT", [128, 2], F32, isOutput=False)
    onesD = nc.declare_dram_parameter("onesD", [W, W], F32R, isOutput=False)
    out = nc.declare_dram_parameter("out", [B * Q, H], F32, isOutput=True)

    with tile.TileContext(nc) as tc:
        with (
            tc.tile_pool(name="const", bufs=1) as cpool,
            tc.tile_pool(name="dec", bufs=4) as dec_pool,
            tc.tile_pool(name="decb", bufs=4) as decb_pool,
            tc.tile_pool(name="sm", bufs=2) as sm_pool,
            tc.tile_pool(name="vst", bufs=2) as vst_pool,
            tc.tile_pool(name="outp", bufs=2) as out_pool,
            tc.tile_pool(name="psS", bufs=2, space="PSUM") as psS,
            tc.tile_pool(name="psB", bufs=6, space="PSUM") as psB,
        ):
            # ---------------- resident tensors ----------------
            enc_sb = cpool.tile([128, HC, B * W], F32R)
            encb_sb = cpool.tile([128, HC, B * W], BF16)
            WaT_sb = cpool.tile([128, HC, H], F32R)
            Wc1_sb = cpool.tile([128, HC, H], BF16)
            Wc2_sb = cpool.tile([128, HC, H], BF16)
            uT_sb = cpool.tile([128, HC, B * W], F32R)
            v_sb = cpool.tile([W, B, H], BF16)
            bias_sb = cpool.tile([W, B], F32)
            gpack_sb = cpool.tile([128, 2], F32)
            ones_sb = cpool.tile([W, W], F32R)
            warm_sb = cpool.tile([128, 64], BF16)

            enc_r = enc_wT[:, :].rearrange("(c p) m -> p c m", p=128)
            encb_r = enc_wTb[:, :].rearrange("(c p) m -> p c m", p=128)
            WaT_r = W_aT[:, :].rearrange("(c p) m -> p c m", p=128)
            Wc1_r = W_c1T[:, :].rearrange("(c p) m -> p c m", p=128)
            Wc2_r = W_c2T[:, :].rearrange("(c p) m -> p c m", p=128)
            dec_r = dec_hT[:, :].rearrange("(c p) (n q) -> p c n q", p=128, q=Q)

            # ---------------- DMA schedule (issue order = priority) --------
            # tiny constants first (scalar ring)
            nc.scalar.dma_start(out=bias_sb, in_=biasT[:, :])
            nc.scalar.dma_start(out=gpack_sb, in_=gPackT[:, :])
            nc.scalar.dma_start(out=ones_sb, in_=onesD[:, :])

            # u deps kc-major: enc chunk + W_aT lo-half per kc, both queues
            for kc in range(HC):
                e1, e2 = (nc.sync, nc.scalar) if kc % 2 == 0 else (nc.scalar, nc.sync)
                e1.dma_start(out=WaT_sb[:, kc, 0:512], in_=WaT_r[:, kc, 0:512])
                e2.dma_start(out=enc_sb[:, kc, :], in_=enc_r[:, kc, :])
            for kc in range(HC):
                eng = nc.sync if kc % 2 == 0 else nc.scalar
                eng.dma_start(out=WaT_sb[:, kc, 512:1024], in_=WaT_r[:, kc, 512:1024])

            # dec0 (score of batch 0 follows u directly)
            dec_tiles = {}

            def load_dec(n):
                dt_ = dec_pool.tile([128, HC, Q], F32R, tag="dec", name=f"dec{n}")
                e1, e2 = (nc.sync, nc.scalar) if n % 2 == 0 else (nc.scalar, nc.sync)
                e1.dma_start(out=dt_[:, 0:4, :], in_=dec_r[:, 0:4, n, :])
                e2.dma_start(out=dt_[:, 4:8, :], in_=dec_r[:, 4:8, n, :])
                dec_tiles[n] = dt_

            load_dec(0)

            # v deps: bf16 windows + W_c1T (nt0 half first, matching v order)
            nc.sync.dma_start(out=encb_sb[:, 0:4, :], in_=encb_r[:, 0:4, :])
            nc.scalar.dma_start(out=encb_sb[:, 4:8, :], in_=encb_r[:, 4:8, :])
            nc.sync.dma_start(out=Wc1_sb[:, 0:4, 0:512], in_=Wc1_r[:, 0:4, 0:512])
            nc.scalar.dma_start(out=Wc1_sb[:, 4:8, 0:512], in_=Wc1_r[:, 4:8, 0:512])
            nc.sync.dma_start(out=Wc1_sb[:, 0:4, 512:1024], in_=Wc1_r[:, 0:4, 512:1024])
            nc.scalar.dma_start(out=Wc1_sb[:, 4:8, 512:1024], in_=Wc1_r[:, 4:8, 512:1024])

            # W_c2T ht0 half, dec1, W_c2T ht1 half, dec2, then dec3+ on demand
            nc.sync.dma_start(out=Wc2_sb[:, 0:4, 0:512], in_=Wc2_r[:, 0:4, 0:512])
            nc.scalar.dma_start(out=Wc2_sb[:, 4:8, 0:512], in_=Wc2_r[:, 4:8, 0:512])
            load_dec(1)
            nc.sync.dma_start(out=Wc2_sb[:, 0:4, 512:1024], in_=Wc2_r[:, 0:4, 512:1024])
            nc.scalar.dma_start(out=Wc2_sb[:, 4:8, 512:1024], in_=Wc2_r[:, 4:8, 512:1024])
            load_dec(2)

            # ---------------- PE warm-up ----------------
            nc.vector.memset(warm_sb[:, :], 0.0)
            for i in range(N_WARM):
                pw = psS.tile([W, 64], F32, tag="S", name=f"warm{i}")
                nc.tensor.matmul(
                    pw, lhsT=warm_sb[:, 0:W], rhs=warm_sb[:, :], start=True, stop=True
                )

            # ---------------- u phase: uT[h, (n,j)], kc-major waves --------
            for wave in range(2):
                pu = {}
                for kc in range(HC):
                    for ho in range(4):
                        hco = wave * 4 + ho
                        if kc == 0:
                            pu[hco] = psB.tile(
                                [128, B * W], F32, tag="B", name=f"pu{hco}"
                            )
                        nc.tensor.matmul(
                            pu[hco],
                            lhsT=WaT_sb[:, kc, hco * 128:(hco + 1) * 128],
                            rhs=enc_sb[:, kc, :],
                            start=(kc == 0),
                            stop=(kc == HC - 1),
                        )
                for ho in range(4):
                    hco = wave * 4 + ho
                    nc.scalar.copy(out=uT_sb[:, hco, :], in_=pu[hco])

            # ---------------- helpers ----------------
            scored = {}

            def score_part(n):
                dec_sb = dec_tiles[n]
                ps = psS.tile([W, Q], F32, tag="S", name=f"ps{n}")
                for hc in range(HC):
                    nc.tensor.matmul(
                        ps,
                        lhsT=uT_sb[:, hc, n * W:(n + 1) * W],
                        rhs=dec_sb[:, hc, :],
                        start=(hc == 0),
                        stop=(hc == HC - 1),
                    )
                t = sm_pool.tile([W, Q], F32R, tag="t", name=f"t{n}")
                nc.scalar.activation(
                    out=t, in_=ps, func=AF.Exp, bias=bias_sb[:, n:n + 1], scale=0.25
                )
                # bf16 copy of dec for the W_c2 matmul (DVE cast)
                db = decb_pool.tile([128, HC, Q], BF16, tag="decb", name=f"decb{n}")
                nc.vector.tensor_scalar_mul(db, dec_sb, 1.0)
                scored[n] = (db, t)

            def v_group(g):
                for nt in range(2):
                    pv = psB.tile([128, 512], F32, tag="B", name=f"pv{g}_{nt}")
                    for kc in range(HC):
                        nc.tensor.matmul(
                            pv,
                            lhsT=encb_sb[:, kc, g * 128:(g + 1) * 128],
                            rhs=Wc1_sb[:, kc, nt * 512:(nt + 1) * 512],
                            start=(kc == 0),
                            stop=(kc == HC - 1),
                        )
                    vst = vst_pool.tile([128, 512], BF16, tag="vst", name=f"vst{g}_{nt}")
                    nc.vector.tensor_scalar_mul(vst, pv, gpack_sb[:, g:g + 1])
                    for o in range(4):
                        nb = g * 4 + o
                        eng = nc.sync if (o + nt) % 2 == 0 else nc.scalar
                        eng.dma_start(
                            out=v_sb[:, nb, nt * 512:(nt + 1) * 512],
                            in_=vst[o * W:(o + 1) * W, :],
                        )

            prev = None  # (n, pos, o_sb) awaiting tanh + store

            def flush_prev():
                nonlocal prev
                if prev is None:
                    return
                pn, ppos, po_sb = prev
                for qt in range(2):
                    for ht in range(2):
                        nc.scalar.activation(
                            out=po_sb[:, qt, ht * 512:(ht + 1) * 512],
                            in_=ppos[(qt, ht)], func=AF.Tanh,
                        )
                dst = out[pn * Q:(pn + 1) * Q, :].rearrange("(qt p) h -> p qt h", p=128)
                nc.sync.dma_start(out=dst[:, 0, :], in_=po_sb[:, 0, :])
                nc.scalar.dma_start(out=dst[:, 1, :], in_=po_sb[:, 1, :])
                prev = None

            state = {}

            def batch_pre(n):
                if 3 <= n + 2 < B:
                    load_dec(n + 2)
                if n not in scored:
                    score_part(n)
                db, t = scored.pop(n)
                flush_prev()
                o_sb = out_pool.tile([128, 2, H], F32, tag="o", name=f"o{n}")
                pos = {}

                def dec_group(qt, ht):
                    po = psB.tile([128, 512], F32, tag="B", name=f"po{n}_{qt}_{ht}")
                    pos[(qt, ht)] = po
                    for hc in range(HC):
                        nc.tensor.matmul(
                            po,
                            lhsT=db[:, hc, qt * 128:(qt + 1) * 128],
                            rhs=Wc2_sb[:, hc, ht * 512:(ht + 1) * 512],
                            start=(hc == 0),
                            stop=False,
                        )

                # ht-outer order so batch 0 only needs the W_c2T ht0 half early
                dec_group(0, 0)
                pT = psS.tile([W, Q], F32, tag="S", name=f"pT{n}")
                nc.tensor.matmul(pT, lhsT=ones_sb[:, :], rhs=t[:, :], start=True, stop=True)
                rT = sm_pool.tile([W, Q], F32, tag="r", name=f"rT{n}")
                nc.vector.reciprocal_approx_fast(out=rT, in_=pT)
                nc.vector.tensor_mul(t, t, rT)
                nc.vector.tensor_mul(t, t, t)
                nc.vector.tensor_mul(t, t, t)
                dec_group(1, 0)
                pZ = psS.tile([W, Q], F32, tag="S", name=f"pZ{n}")
                nc.tensor.matmul(pZ, lhsT=ones_sb[:, :], rhs=t[:, :], start=True, stop=True)
                rZ = sm_pool.tile([W, Q], F32, tag="r", name=f"rZ{n}")
                nc.vector.reciprocal_approx_fast(out=rZ, in_=pZ)
                tb = sm_pool.tile([W, Q], BF16, tag="tb", name=f"tb{n}")
                nc.vector.tensor_mul(tb, t, rZ)
                dec_group(0, 1)
                dec_group(1, 1)
                state[n] = (tb, pos, o_sb)

            def batch_ctx(n):
                tb, pos, o_sb = state.pop(n)
                last = n == B - 1
                dst = out[n * Q:(n + 1) * Q, :].rearrange("(qt p) h -> p qt h", p=128)
                for qt in range(2):
                    for ht in range(2):
                        nc.tensor.matmul(
                            pos[(qt, ht)],
                            lhsT=tb[:, qt * 128:(qt + 1) * 128],
                            rhs=v_sb[:, n, ht * 512:(ht + 1) * 512],
                            start=False,
                            stop=True,
                        )
                        if last:
                            nc.scalar.activation(
                                out=o_sb[:, qt, ht * 512:(ht + 1) * 512],
                                in_=pos[(qt, ht)], func=AF.Tanh,
                            )
                    if last:
                        eng = nc.sync if qt == 0 else nc.scalar
                        eng.dma_start(out=dst[:, qt, :], in_=o_sb[:, qt, :])
                nonlocal prev
                if not last:
                    prev = (n, pos, o_sb)

            # ---------------- main sequence ----------------
            score_part(0)
            v_group(0)
            v_group(1)
            for n in range(B):
                batch_pre(n)
                batch_ctx(n)
            flush_prev()
    nc.compile()
    return nc


def round_f32r(a: np.ndarray) -> np.ndarray:
    """Round fp32 to fp32r (TF32-like: 11-bit mantissa, low 12 bits zero),
    round-to-nearest-even.  This is what the PE consumes in fp32r mode."""
    u = np.ascontiguousarray(a, dtype=np.float32).view(np.uint32)
    lsb = (u >> np.uint32(12)) & np.uint32(1)
    u = (u + np.uint32(0x7FF) + lsb) & np.uint32(0xFFFFF000)
    return u.view(np.float32)


def prepare_in_maps(inputs: dict) -> list[dict]:
    enc = np.asarray(inputs["encoder_outputs"], dtype=np.float32)
    dec = np.asarray(inputs["decoder_h_t"], dtype=np.float32)
    src_len = np.asarray(inputs["src_len"], dtype=np.int32)
    p_t = np.asarray(inputs["p_t"], dtype=np.float32)
    W_a = np.asarray(inputs["W_a"], dtype=np.float32)
    W_c = np.asarray(inputs["W_c"], dtype=np.float32)

    # Window bounds, computed with the same fp32 ops as the reference.
    attn_start = np.maximum(p_t - np.float32(WINDOW), np.float32(0.0))
    attn_end = np.minimum(p_t + np.float32(WINDOW), src_len.astype(np.float32))
    idx_full = np.arange(L, dtype=np.float32)
    mask_full = (idx_full[None, :] < attn_start[:, None]) | (
        idx_full[None, :] > attn_end[:, None]
    )
    live = ~mask_full
    s = np.clip(live.argmax(axis=1), 0, L - W)  # first live position per batch
    idx = s[:, None] + np.arange(W)[None, :]
    idxf = idx.astype(np.float32)
    mask = (idxf < attn_start[:, None]) | (idxf > attn_end[:, None])
    bias = np.where(mask, np.float32(MASK_BIAS), np.float32(LOG_ALPHA)).astype(np.float32)
    g = np.exp(-((idxf - p_t[:, None]) ** 2) / np.float32(DEV_POW)).astype(np.float32)

    enc_w = round_f32r(enc[np.arange(N)[:, None], idx, :])  # [N, W, H]
    dec = round_f32r(dec)
    W_aT = round_f32r(W_a.T)
    W_c1Tb = W_c[:, :H].T.astype(ml_dtypes.bfloat16)
    W_c2Tb = W_c[:, H:].T.astype(ml_dtypes.bfloat16)

    in_maps = []
    for c in range(NCORES):
        bs = slice(c * B, (c + 1) * B)
        gc = g[bs]  # [B, W]
        gpack = np.zeros((128, 2), dtype=np.float32)
        for n in range(B):
            gi, off = divmod(n, 4)
            gpack[off * W:(off + 1) * W, gi] = gc[n]
        enc_wT = np.ascontiguousarray(enc_w[bs].transpose(2, 0, 1).reshape(H, B * W))
        in_maps.append({
            "enc_wT": enc_wT,
            "enc_wTb": enc_wT.astype(ml_dtypes.bfloat16),
            "dec_hT": np.ascontiguousarray(dec[bs].transpose(2, 0, 1).reshape(H, B * Q)),
            "W_aT": W_aT,
            "W_c1T": W_c1Tb,
            "W_c2T": W_c2Tb,
            "biasT": np.ascontiguousarray(bias[bs].T),
            "onesD": np.ones((W, W), dtype=np.float32),
            "gPackT": gpack,
        })
    return in_maps


_NC = None


def get_nc() -> bass.Bass:
    global _NC
    if _NC is None:
        _NC = build_nc()
    return _NC


def kernel(**inputs) -> np.ndarray:
    nc = get_nc()
    in_maps = prepare_in_maps(inputs)
    res = run_bass_kernel_spmd(nc, in_maps, list(range(NCORES)))
    outs = [res.results[c]["out"].reshape(B, Q, H) for c in range(NCORES)]
    return np.concatenate(outs, axis=0)


# revision 7
# speedup vs baseline: 1.0502x; 1.0502x over previous
"""Trainium2 Bass kernel for Luong local-p sparse attention (v2).

Math (per batch n, full shapes N=64, L=258, H=1024, Q=256):
    score = (h_t @ W_a) @ enc^T           masked to window [p_t-16, p_t+16]
    align = softmax(score) * gauss(p_t)
    out   = tanh([align @ enc, h_t] @ W_c^T)

Only a 32-wide slice of enc can survive the mask for non-integer p_t, so the
kernel gathers 32-wide windows host-side (W=32 -> 4 windows pack exactly into
128 PE partitions).  Score path (u = W_a-transform of windows, score, softmax)
runs in fp32r for softmax accuracy; the heavy W_c matmuls (dec @ W_c2T, window
@ W_c1T, align @ v) run in bf16 (same PE rate, half the DMA/SBUF).

    u  = W_a-transform of window   (uT[h, (n,j)]  = sum_k W_aT[k,h] enc_w[(n,j),k])
    s  = uT^T-partial scores       (score^T[j, q] = sum_h uT[h,j] h_t[q,h])
    softmax over j (32 rows) j-major with a 4th-power renormalization trick
    v  = W_c1-transform of window  (bf16), gaussian folded on PSUM evacuation
    out = tanh(dec_b16 @ W_c2T_b16 + align_b16^T.T @ v)

Schedule: PE warm-up matmuls cover the initial DMA window (keeps the HAM
clock gate at 8/8); u phase streams kc-major behind the enc/W_aT DMA; v and
batch 0 overlap the W_c1/W_c2/dec streams; batches pipeline as in the
baseline with dec prefetched 2 batches ahead.  Data parallel over batch:
8 batches per core x 8 cores.
"""

import numpy as np
import ml_dtypes

import concourse.bass as bass
import concourse.bacc as bacc
import concourse.mybir as mybir
import concourse.tile as tile
from concourse.bass_utils import run_bass_kernel_spmd

# Problem constants (hardcoded per harness contract).
N, L, H, Q = 64, 258, 1024, 256
WINDOW = 16.0
DEV_POW = 128.0
NCORES = 8
B = N // NCORES  # batches per core
W = 32           # window width (max live positions for non-integer p_t)
HC = H // 128    # h-chunks of 128 (PE contraction tiles)
F32 = mybir.dt.float32
F32R = mybir.dt.float32r
BF16 = mybir.dt.bfloat16
AF = mybir.ActivationFunctionType

# exp is computed as t = exp(s/4 + bias); bias = LOG_ALPHA keeps the
# column-sum T = sum_j t below fp32 max.  alpha cancels in w = t/T.
LOG_ALPHA = -4.8520302  # -7*ln(2)
MASK_BIAS = -10000.0    # exp(<= -2500) == 0 in fp32
N_WARM = 36             # PE warm-up matmuls spanning the initial DMA wait


def build_nc() -> bass.Bass:
    nc = bacc.Bacc()
    enc_wT = nc.declare_dram_parameter("enc_wT", [H, B * W], F32R, isOutput=False)
    enc_wTb = nc.declare_dram_parameter("enc_wTb", [H, B * W], BF16, isOutput=False)
    dec_hT = nc.declare_dram_parameter("dec_hT", [H, B * Q], F32R, isOutput=False)
    W_aT = nc.declare_dram_parameter("W_aT", [H, H], F32R, isOutput=False)
    W_c1T = nc.declare_dram_parameter("W_c1T", [H, H], BF16, isOutput=False)
    W_c2T = nc.declare_dram_parameter("W_c2T", [H, H], BF16, isOutput=False)
    biasT = nc.declare_dram_parameter("biasT", [W, B], F32, isOutput=False)
    gPackT = nc.declare_dram_parameter("gPackT", [128, 2], F32, isOutput=False)
    onesD = nc.declare_dram_parameter("onesD", [W, W], F32R, isOutput=False)
    out = nc.declare_dram_parameter("out", [B * Q, H], F32, isOutput=True)

    with tile.TileContext(nc) as tc:
        with (
            tc.tile_pool(name="const", bufs=1) as cpool,
            tc.tile_pool(name="dec", bufs=4) as dec_pool,
            tc.tile_pool(name="decb", bufs=4) as decb_pool,
            tc.tile_pool(name="sm", bufs=2) as sm_pool,
            tc.tile_pool(name="vst", bufs=2) as vst_pool,
            tc.tile_pool(name="outp", bufs=2) as out_pool,
            tc.tile_pool(name="psS", bufs=2, space="PSUM") as psS,
            tc.tile_pool(name="psB", bufs=6, space="PSUM") as psB,
        ):
            # ---------------- resident tensors ----------------
            enc_sb = cpool.tile([128, HC, B * W], F32R)
            encb_sb = cpool.tile([128, HC, B * W], BF16)
            WaT_sb = cpool.tile([128, HC, H], F32R)
            Wc1_sb = cpool.tile([128, HC, H], BF16)
            Wc2_sb = cpool.tile([128, HC, H], BF16)
            uT_sb = cpool.tile([128, HC, B * W], F32R)
            v_sb = cpool.tile([W, B, H], BF16)
            bias_sb = cpool.tile([W, B], F32)
            gpack_sb = cpool.tile([128, 2], F32)
            ones_sb = cpool.tile([W, W], F32R)
            warm_sb = cpool.tile([128, 640], BF16)

            enc_r = enc_wT[:, :].rearrange("(c p) m -> p c m", p=128)
            encb_r = enc_wTb[:, :].rearrange("(c p) m -> p c m", p=128)
            WaT_r = W_aT[:, :].rearrange("(c p) m -> p c m", p=128)
            Wc1_r = W_c1T[:, :].rearrange("(c p) m -> p c m", p=128)
            Wc2_r = W_c2T[:, :].rearrange("(c p) m -> p c m", p=128)
            dec_r = dec_hT[:, :].rearrange("(c p) (n q) -> p c n q", p=128, q=Q)

            # ---------------- DMA schedule (issue order = priority) --------
            # tiny constants first (scalar ring)
            nc.scalar.dma_start(out=bias_sb, in_=biasT[:, :])
            nc.scalar.dma_start(out=gpack_sb, in_=gPackT[:, :])
            nc.scalar.dma_start(out=ones_sb, in_=onesD[:, :])

            # u deps first, in coarse linear chunks (full dram rows -> max BW):
            # enc halves + W_aT kc-pair chunks, alternating queues, kc-major
            nc.sync.dma_start(out=enc_sb[:, 0:4, :], in_=enc_r[:, 0:4, :])
            nc.scalar.dma_start(out=WaT_sb[:, 0:2, :], in_=WaT_r[:, 0:2, :])
            nc.sync.dma_start(out=enc_sb[:, 4:8, :], in_=enc_r[:, 4:8, :])
            nc.scalar.dma_start(out=WaT_sb[:, 2:4, :], in_=WaT_r[:, 2:4, :])
            nc.sync.dma_start(out=WaT_sb[:, 4:6, :], in_=WaT_r[:, 4:6, :])
            nc.scalar.dma_start(out=WaT_sb[:, 6:8, :], in_=WaT_r[:, 6:8, :])

            # dec0 (score of batch 0 follows u directly)
            dec_tiles = {}

            def load_dec(n):
                dt_ = dec_pool.tile([128, HC, Q], F32R, tag="dec", name=f"dec{n}")
                eng = nc.sync if n % 2 == 0 else nc.scalar
                eng.dma_start(out=dt_, in_=dec_r[:, :, n, :])
                dec_tiles[n] = dt_

            load_dec(0)

            # v deps: bf16 windows + W_c1T halves (linear chunks)
            nc.sync.dma_start(out=encb_sb, in_=encb_r[:, :, :])
            nc.scalar.dma_start(out=Wc1_sb[:, 0:4, :], in_=Wc1_r[:, 0:4, :])
            nc.sync.dma_start(out=Wc1_sb[:, 4:8, :], in_=Wc1_r[:, 4:8, :])

            # W_c2T halves, dec1..3; dec4+ prefetched inside the batch loop
            nc.scalar.dma_start(out=Wc2_sb[:, 0:4, :], in_=Wc2_r[:, 0:4, :])
            nc.sync.dma_start(out=Wc2_sb[:, 4:8, :], in_=Wc2_r[:, 4:8, :])
            load_dec(1)
            load_dec(2)
            load_dec(3)

            # ---------------- PE warm-up ----------------
            # Long back-to-back matmuls cycling all 6 big PSUM slots (deep
            # pipelining hides slot-reuse semaphores); sustained PE busy trips
            # the HAM clock gate to 8/8 before real work starts and holds it.
            nc.vector.memset(warm_sb[:, :], 1.0)
            for i in range(N_WARM):
                pw = psB.tile([128, 512], F32, tag="B", name=f"warm{i}")
                nc.tensor.matmul(
                    pw, lhsT=warm_sb[:, 0:128], rhs=warm_sb[:, 128:640],
                    start=True, stop=True,
                )

            # ---------------- u phase: uT[h, (n,j)], kc-major waves --------
            for wave in range(2):
                pu = {}
                for kc in range(HC):
                    for ho in range(4):
                        hco = wave * 4 + ho
                        if kc == 0:
                            pu[hco] = psB.tile(
                                [128, B * W], F32, tag="B", name=f"pu{hco}"
                            )
                        nc.tensor.matmul(
                            pu[hco],
                            lhsT=WaT_sb[:, kc, hco * 128:(hco + 1) * 128],
                            rhs=enc_sb[:, kc, :],
                            start=(kc == 0),
                            stop=(kc == HC - 1),
                        )
                for ho in range(4):
                    hco = wave * 4 + ho
                    nc.scalar.copy(out=uT_sb[:, hco, :], in_=pu[hco])

            # ---------------- helpers ----------------
            scored = {}

            def score_part(n):
                dec_sb = dec_tiles[n]
                ps = psS.tile([W, Q], F32, tag="S", name=f"ps{n}")
                for hc in range(HC):
                    nc.tensor.matmul(
                        ps,
                        lhsT=uT_sb[:, hc, n * W:(n + 1) * W],
                        rhs=dec_sb[:, hc, :],
                        start=(hc == 0),
                        stop=(hc == HC - 1),
                    )
                t = sm_pool.tile([W, Q], F32R, tag="t", name=f"t{n}")
                nc.scalar.activation(
                    out=t, in_=ps, func=AF.Exp, bias=bias_sb[:, n:n + 1], scale=0.25
                )
                # bf16 copy of dec for the W_c2 matmul (DVE cast)
                db = decb_pool.tile([128, HC, Q], BF16, tag="decb", name=f"decb{n}")
                nc.vector.tensor_scalar_mul(db, dec_sb, 1.0)
                scored[n] = (db, t)

            def v_group(g):
                for nt in range(2):
                    pv = psB.tile([128, 512], F32, tag="B", name=f"pv{g}_{nt}")
                    for kc in range(HC):
                        nc.tensor.matmul(
                            pv,
                            lhsT=encb_sb[:, kc, g * 128:(g + 1) * 128],
                            rhs=Wc1_sb[:, kc, nt * 512:(nt + 1) * 512],
                            start=(kc == 0),
                            stop=(kc == HC - 1),
                        )
                    vst = vst_pool.tile([128, 512], BF16, tag="vst", name=f"vst{g}_{nt}")
                    nc.vector.tensor_scalar_mul(vst, pv, gpack_sb[:, g:g + 1])
                    for o in range(4):
                        nb = g * 4 + o
                        eng = nc.sync if (o + nt) % 2 == 0 else nc.scalar
                        eng.dma_start(
                            out=v_sb[:, nb, nt * 512:(nt + 1) * 512],
                            in_=vst[o * W:(o + 1) * W, :],
                        )

            prev = None  # (n, pos, o_sb) awaiting tanh + store

            def flush_prev():
                nonlocal prev
                if prev is None:
                    return
                pn, ppos, po_sb = prev
                for qt in range(2):
                    for ht in range(2):
                        nc.scalar.activation(
                            out=po_sb[:, qt, ht * 512:(ht + 1) * 512],
                            in_=ppos[(qt, ht)], func=AF.Tanh,
                        )
                dst = out[pn * Q:(pn + 1) * Q, :].rearrange("(qt p) h -> p qt h", p=128)
                nc.sync.dma_start(out=dst[:, 0, :], in_=po_sb[:, 0, :])
                nc.scalar.dma_start(out=dst[:, 1, :], in_=po_sb[:, 1, :])
                prev = None

            state = {}

            def batch_pre(n):
                if 4 <= n + 2 < B:
                    load_dec(n + 2)
                if n not in scored:
                    score_part(n)
                db, t = scored.pop(n)
                flush_prev()
                o_sb = out_pool.tile([128, 2, H], F32, tag="o", name=f"o{n}")
                pos = {}

                def dec_group(qt, ht):
                    po = psB.tile([128, 512], F32, tag="B", name=f"po{n}_{qt}_{ht}")
                    pos[(qt, ht)] = po
                    for hc in range(HC):
                        nc.tensor.matmul(
                            po,
                            lhsT=db[:, hc, qt * 128:(qt + 1) * 128],
                            rhs=Wc2_sb[:, hc, ht * 512:(ht + 1) * 512],
                            start=(hc == 0),
                            stop=False,
                        )

                # ht-outer order so batch 0 only needs the W_c2T ht0 half early
                dec_group(0, 0)
                pT = psS.tile([W, Q], F32, tag="S", name=f"pT{n}")
                nc.tensor.matmul(pT, lhsT=ones_sb[:, :], rhs=t[:, :], start=True, stop=True)
                rT = sm_pool.tile([W, Q], F32, tag="r", name=f"rT{n}")
                nc.vector.reciprocal_approx_fast(out=rT, in_=pT)
                nc.vector.tensor_mul(t, t, rT)
                nc.vector.tensor_mul(t, t, t)
                nc.vector.tensor_mul(t, t, t)
                dec_group(1, 0)
                pZ = psS.tile([W, Q], F32, tag="S", name=f"pZ{n}")
                nc.tensor.matmul(pZ, lhsT=ones_sb[:, :], rhs=t[:, :], start=True, stop=True)
                rZ = sm_pool.tile([W, Q], F32, tag="r", name=f"rZ{n}")
                nc.vector.reciprocal_approx_fast(out=rZ, in_=pZ)
                tb = sm_pool.tile([W, Q], BF16, tag="tb", name=f"tb{n}")
                nc.vector.tensor_mul(tb, t, rZ)
                dec_group(0, 1)
                dec_group(1, 1)
                state[n] = (tb, pos, o_sb)

            def batch_ctx(n):
                tb, pos, o_sb = state.pop(n)
                last = n == B - 1
                dst = out[n * Q:(n + 1) * Q, :].rearrange("(qt p) h -> p qt h", p=128)
                for qt in range(2):
                    for ht in range(2):
                        nc.tensor.matmul(
                            pos[(qt, ht)],
                            lhsT=tb[:, qt * 128:(qt + 1) * 128],
                            rhs=v_sb[:, n, ht * 512:(ht + 1) * 512],
                            start=False,
                            stop=True,
                        )
                        if last:
                            nc.scalar.activation(
                                out=o_sb[:, qt, ht * 512:(ht + 1) * 512],
                                in_=pos[(qt, ht)], func=AF.Tanh,
                            )
                    if last:
                        eng = nc.sync if qt == 0 else nc.scalar
                        eng.dma_start(out=dst[:, qt, :], in_=o_sb[:, qt, :])
                nonlocal prev
                if not last:
                    prev = (n, pos, o_sb)

            # ---------------- main sequence ----------------
            score_part(0)
            v_group(0)
            v_group(1)
            for n in range(B):
                batch_pre(n)
                batch_ctx(n)
            flush_prev()
    nc.compile()
    return nc


def round_f32r(a: np.ndarray) -> np.ndarray:
    """Round fp32 to fp32r (TF32-like: 11-bit mantissa, low 12 bits zero),
    round-to-nearest-even.  This is what the PE consumes in fp32r mode."""
    u = np.ascontiguousarray(a, dtype=np.float32).view(np.uint32)
    lsb = (u >> np.uint32(12)) & np.uint32(1)
    u = (u + np.uint32(0x7FF) + lsb) & np.uint32(0xFFFFF000)
    return u.view(np.float32)


def prepare_in_maps(inputs: dict) -> list[dict]:
    enc = np.asarray(inputs["encoder_outputs"], dtype=np.float32)
    dec = np.asarray(inputs["decoder_h_t"], dtype=np.float32)
    src_len = np.asarray(inputs["src_len"], dtype=np.int32)
    p_t = np.asarray(inputs["p_t"], dtype=np.float32)
    W_a = np.asarray(inputs["W_a"], dtype=np.float32)
    W_c = np.asarray(inputs["W_c"], dtype=np.float32)

    # Window bounds, computed with the same fp32 ops as the reference.
    attn_start = np.maximum(p_t - np.float32(WINDOW), np.float32(0.0))
    attn_end = np.minimum(p_t + np.float32(WINDOW), src_len.astype(np.float32))
    idx_full = np.arange(L, dtype=np.float32)
    mask_full = (idx_full[None, :] < attn_start[:, None]) | (
        idx_full[None, :] > attn_end[:, None]
    )
    live = ~mask_full
    s = np.clip(live.argmax(axis=1), 0, L - W)  # first live position per batch
    idx = s[:, None] + np.arange(W)[None, :]
    idxf = idx.astype(np.float32)
    mask = (idxf < attn_start[:, None]) | (idxf > attn_end[:, None])
    bias = np.where(mask, np.float32(MASK_BIAS), np.float32(LOG_ALPHA)).astype(np.float32)
    g = np.exp(-((idxf - p_t[:, None]) ** 2) / np.float32(DEV_POW)).astype(np.float32)

    enc_w = round_f32r(enc[np.arange(N)[:, None], idx, :])  # [N, W, H]
    dec = round_f32r(dec)
    W_aT = round_f32r(W_a.T)
    W_c1Tb = W_c[:, :H].T.astype(ml_dtypes.bfloat16)
    W_c2Tb = W_c[:, H:].T.astype(ml_dtypes.bfloat16)

    in_maps = []
    for c in range(NCORES):
        bs = slice(c * B, (c + 1) * B)
        gc = g[bs]  # [B, W]
        gpack = np.zeros((128, 2), dtype=np.float32)
        for n in range(B):
            gi, off = divmod(n, 4)
            gpack[off * W:(off + 1) * W, gi] = gc[n]
        enc_wT = np.ascontiguousarray(enc_w[bs].transpose(2, 0, 1).reshape(H, B * W))
        in_maps.append({
            "enc_wT": enc_wT,
            "enc_wTb": enc_wT.astype(ml_dtypes.bfloat16),
            "dec_hT": np.ascontiguousarray(dec[bs].transpose(2, 0, 1).reshape(H, B * Q)),
            "W_aT": W_aT,
            "W_c1T": W_c1Tb,
            "W_c2T": W_c2Tb,
            "biasT": np.ascontiguousarray(bias[bs].T),
            "onesD": np.ones((W, W), dtype=np.float32),
            "gPackT": gpack,
        })
    return in_maps


_NC = None


def get_nc() -> bass.Bass:
    global _NC
    if _NC is None:
        _NC = build_nc()
    return _NC


def kernel(**inputs) -> np.ndarray:
    nc = get_nc()
    in_maps = prepare_in_maps(inputs)
    res = run_bass_kernel_spmd(nc, in_maps, list(range(NCORES)))
    outs = [res.results[c]["out"].reshape(B, Q, H) for c in range(NCORES)]
    return np.concatenate(outs, axis=0)


# revision 11
# speedup vs baseline: 1.0687x; 1.0177x over previous
"""Trainium2 Bass kernel for Luong local-p sparse attention (v4).

Math (per batch n, full shapes N=64, L=258, H=1024, Q=256):
    score = (h_t @ W_a) @ enc^T           masked to window [p_t-16, p_t+16]
    align = softmax(score) * gauss(p_t)
    out   = tanh([align @ enc, h_t] @ W_c^T)

Only a 32-wide slice of enc can survive the mask for non-integer p_t, so the
kernel gathers 32-wide windows host-side (W=32 -> 4 windows pack exactly into
128 PE partitions).  Score path (u = W_a-transform of windows, score, softmax)
runs in fp32r for softmax accuracy; the heavy W_c matmuls (dec @ W_c2T, window
@ W_c1T, align @ v) run in bf16 (same PE rate, half the DMA/SBUF).

Schedule ("dec_group-first"): the dec @ W_c2T matmuls for batches 0-3 need
only dec + W_c2T (3 MB), so after a short PE warm-up they run PE-bound from
~13us while enc/W_aT/W_c1T stream in behind them; their partials are
evacuated to SBUF (bf16) and added back at context time (DVE add + tanh).
Then u -> scores 0-3 -> v -> contexts 0-3, and batches 4-7 run the standard
softly-pipelined score/dec_group/ctx schedule.  The PE never idles long
enough for the HAM clock gate to drop from 8/8.  Data parallel over batch:
8 batches per core x 8 cores.
"""

import numpy as np
import ml_dtypes

import concourse.bass as bass
import concourse.bacc as bacc
import concourse.mybir as mybir
import concourse.tile as tile
from concourse.bass_utils import run_bass_kernel_spmd

# Problem constants (hardcoded per harness contract).
N, L, H, Q = 64, 258, 1024, 256
WINDOW = 16.0
DEV_POW = 128.0
NCORES = 8
B = N // NCORES  # batches per core
W = 32           # window width (max live positions for non-integer p_t)
HC = H // 128    # h-chunks of 128 (PE contraction tiles)
F32 = mybir.dt.float32
F32R = mybir.dt.float32r
BF16 = mybir.dt.bfloat16
AF = mybir.ActivationFunctionType

# exp is computed as t = exp(s/4 + bias); bias = LOG_ALPHA keeps the
# column-sum T = sum_j t below fp32 max.  alpha cancels in w = t/T.
LOG_ALPHA = -4.8520302  # -7*ln(2)
MASK_BIAS = -10000.0    # exp(<= -2500) == 0 in fp32
N_WARM = 32             # PE warm-up matmuls spanning the initial DMA wait
NB_EARLY = 4            # batches whose dec_group runs ahead of u/score/v


def build_nc() -> bass.Bass:
    nc = bacc.Bacc()
    enc_wT = nc.declare_dram_parameter("enc_wT", [H, B * W], F32R, isOutput=False)
    enc_wTb = nc.declare_dram_parameter("enc_wTb", [H, B * W], BF16, isOutput=False)
    dec_hT = nc.declare_dram_parameter("dec_hT", [H, B * Q], F32R, isOutput=False)
    W_aT = nc.declare_dram_parameter("W_aT", [H, H], F32R, isOutput=False)
    W_c1T = nc.declare_dram_parameter("W_c1T", [H, H], BF16, isOutput=False)
    W_c2T = nc.declare_dram_parameter("W_c2T", [H, H], BF16, isOutput=False)
    biasT = nc.declare_dram_parameter("biasT", [W, B], F32, isOutput=False)
    gPackT = nc.declare_dram_parameter("gPackT", [128, 2], F32, isOutput=False)
    onesD = nc.declare_dram_parameter("onesD", [W, W], F32R, isOutput=False)
    out = nc.declare_dram_parameter("out", [B * Q, H], F32, isOutput=True)

    with tile.TileContext(nc) as tc:
        with (
            tc.tile_pool(name="const", bufs=1) as cpool,
            tc.tile_pool(name="dec", bufs=4) as dec_pool,
            tc.tile_pool(name="decb", bufs=4) as decb_pool,
            tc.tile_pool(name="dgp", bufs=NB_EARLY) as dgp_pool,
            tc.tile_pool(name="sm", bufs=2) as sm_pool,
            tc.tile_pool(name="vst", bufs=2) as vst_pool,
            tc.tile_pool(name="outp", bufs=2) as out_pool,
            tc.tile_pool(name="psS", bufs=2, space="PSUM") as psS,
            tc.tile_pool(name="psB", bufs=6, space="PSUM") as psB,
        ):
            # ---------------- resident tensors ----------------
            enc_sb = cpool.tile([128, HC, B * W], F32R)
            encb_sb = cpool.tile([128, HC, B * W], BF16)
            WaT_sb = cpool.tile([128, HC, H], F32R)
            Wc1_sb = cpool.tile([128, HC, H], BF16)
            Wc2_sb = cpool.tile([128, HC, H], BF16)
            uT_sb = cpool.tile([128, HC, B * W], F32R)
            v_sb = cpool.tile([W, B, H], BF16)
            bias_sb = cpool.tile([W, B], F32)
            gpack_sb = cpool.tile([128, 2], F32)
            ones_sb = cpool.tile([W, W], F32R)
            warm_sb = cpool.tile([128, 640], BF16)

            enc_r = enc_wT[:, :].rearrange("(c p) m -> p c m", p=128)
            encb_r = enc_wTb[:, :].rearrange("(c p) m -> p c m", p=128)
            WaT_r = W_aT[:, :].rearrange("(c p) m -> p c m", p=128)
            Wc1_r = W_c1T[:, :].rearrange("(c p) m -> p c m", p=128)
            Wc2_r = W_c2T[:, :].rearrange("(c p) m -> p c m", p=128)
            dec_r = dec_hT[:, :].rearrange("(c p) (n q) -> p c n q", p=128, q=Q)

            dec_tiles = {}
            decb_tiles = {}

            def load_dec(n, convert=True):
                dt_ = dec_pool.tile([128, HC, Q], F32R, tag="dec", name=f"dec{n}")
                nc.sync.dma_start(out=dt_[:, 0:4, :], in_=dec_r[:, 0:4, n, :])
                nc.scalar.dma_start(out=dt_[:, 4:8, :], in_=dec_r[:, 4:8, n, :])
                dec_tiles[n] = dt_
                if convert:
                    convert_dec(n)

            def convert_dec(n):
                db = decb_pool.tile([128, HC, Q], BF16, tag="decb", name=f"decb{n}")
                nc.vector.tensor_copy(out=db, in_=dec_tiles[n])
                decb_tiles[n] = db

            # ---------------- DMA schedule (issue order = priority) --------
            # tiny constants first (scalar ring)
            nc.scalar.dma_start(out=bias_sb, in_=biasT[:, :])
            nc.scalar.dma_start(out=gpack_sb, in_=gPackT[:, :])
            nc.scalar.dma_start(out=ones_sb, in_=onesD[:, :])

            # dec_group-first deps: dec0, W_c2T, dec1-3 (both queues each)
            load_dec(0)
            nc.sync.dma_start(out=Wc2_sb[:, 0:4, :], in_=Wc2_r[:, 0:4, :])
            nc.scalar.dma_start(out=Wc2_sb[:, 4:8, :], in_=Wc2_r[:, 4:8, :])
            for n in range(1, NB_EARLY):
                load_dec(n)

            # u deps: enc + W_aT (kc-major pairs, both queues)
            nc.sync.dma_start(out=enc_sb[:, 0:4, :], in_=enc_r[:, 0:4, :])
            nc.scalar.dma_start(out=enc_sb[:, 4:8, :], in_=enc_r[:, 4:8, :])
            nc.sync.dma_start(out=WaT_sb[:, 0:2, :], in_=WaT_r[:, 0:2, :])
            nc.scalar.dma_start(out=WaT_sb[:, 2:4, :], in_=WaT_r[:, 2:4, :])
            nc.sync.dma_start(out=WaT_sb[:, 4:6, :], in_=WaT_r[:, 4:6, :])
            nc.scalar.dma_start(out=WaT_sb[:, 6:8, :], in_=WaT_r[:, 6:8, :])

            # v deps: bf16 windows + W_c1T
            nc.sync.dma_start(out=encb_sb, in_=encb_r[:, :, :])
            nc.scalar.dma_start(out=Wc1_sb[:, 0:4, :], in_=Wc1_r[:, 0:4, :])
            nc.sync.dma_start(out=Wc1_sb[:, 4:8, :], in_=Wc1_r[:, 4:8, :])

            # ---------------- PE warm-up ----------------
            # Long back-to-back matmuls cycling all 6 big PSUM slots (deep
            # pipelining hides slot-reuse semaphores); sustained PE busy trips
            # the HAM clock gate to 8/8 before real work starts.
            nc.vector.memset(warm_sb[:, :], 1.0)
            for i in range(N_WARM):
                pw = psB.tile([128, 512], F32, tag="B", name=f"warm{i}")
                nc.tensor.matmul(
                    pw, lhsT=warm_sb[:, 0:128], rhs=warm_sb[:, 128:640],
                    start=True, stop=True,
                )

            # ---------------- helpers ----------------
            def dg_mms(n, qt, ht, po, stop_last=False):
                db = decb_tiles[n]
                for hc in range(HC):
                    nc.tensor.matmul(
                        po,
                        lhsT=db[:, hc, qt * 128:(qt + 1) * 128],
                        rhs=Wc2_sb[:, hc, ht * 512:(ht + 1) * 512],
                        start=(hc == 0),
                        stop=(stop_last and hc == HC - 1),
                    )

            scored = {}

            def score_part(n):
                dec_sb = dec_tiles[n]
                ps = psS.tile([W, Q], F32, tag="S", name=f"ps{n}")
                for hc in range(HC):
                    nc.tensor.matmul(
                        ps,
                        lhsT=uT_sb[:, hc, n * W:(n + 1) * W],
                        rhs=dec_sb[:, hc, :],
                        start=(hc == 0),
                        stop=(hc == HC - 1),
                    )
                t = sm_pool.tile([W, Q], F32R, tag="t", name=f"t{n}")
                nc.scalar.activation(
                    out=t, in_=ps, func=AF.Exp, bias=bias_sb[:, n:n + 1], scale=0.25
                )
                scored[n] = t

            def smx_a(n, t):
                # first renorm pass: T = colsum t; t = (t/T)^4 (two squarings)
                pT = psS.tile([W, Q], F32, tag="S", name=f"pT{n}")
                nc.tensor.matmul(pT, lhsT=ones_sb[:, :], rhs=t[:, :], start=True, stop=True)
                rT = sm_pool.tile([W, Q], F32, tag="r", name=f"rT{n}")
                nc.vector.reciprocal_approx_fast(out=rT, in_=pT)
                nc.vector.tensor_mul(t, t, rT)
                nc.vector.tensor_mul(t, t, t)
                nc.vector.tensor_mul(t, t, t)

            def smx_b(n, t):
                # second renorm pass -> bf16 align weights
                pZ = psS.tile([W, Q], F32, tag="S", name=f"pZ{n}")
                nc.tensor.matmul(pZ, lhsT=ones_sb[:, :], rhs=t[:, :], start=True, stop=True)
                rZ = sm_pool.tile([W, Q], F32, tag="r", name=f"rZ{n}")
                nc.vector.reciprocal_approx_fast(out=rZ, in_=pZ)
                tb = sm_pool.tile([W, Q], BF16, tag="tb", name=f"tb{n}", bufs=NB_EARLY)
                nc.vector.tensor_mul(tb, t, rZ)
                return tb

            def v_group(g):
                for nt in range(2):
                    pv = psB.tile([128, 512], F32, tag="B", name=f"pv{g}_{nt}")
                    for kc in range(HC):
                        nc.tensor.matmul(
                            pv,
                            lhsT=encb_sb[:, kc, g * 128:(g + 1) * 128],
                            rhs=Wc1_sb[:, kc, nt * 512:(nt + 1) * 512],
                            start=(kc == 0),
                            stop=(kc == HC - 1),
                        )
                    vst = vst_pool.tile([128, 512], BF16, tag="vst", name=f"vst{g}_{nt}")
                    nc.vector.tensor_scalar_mul(vst, pv, gpack_sb[:, g:g + 1])
                    # scatter windows via SWDGE (separate DMA path, idle engine)
                    for o in range(4):
                        nb = g * 4 + o
                        nc.gpsimd.dma_start(
                            out=v_sb[:, nb, nt * 512:(nt + 1) * 512],
                            in_=vst[o * W:(o + 1) * W, :],
                        )

            # ---------------- phase 1: dec_group for batches 0-3 ----------
            # Partials evacuated to SBUF bf16; added back at context time.
            dgp_tiles = {}
            for n in range(NB_EARLY):
                dgp = dgp_pool.tile([128, 2, 2, 512], BF16, tag="dgp", name=f"dgp{n}")
                dgp_tiles[n] = dgp
                for qt, ht in ((0, 0), (1, 0), (0, 1), (1, 1)):
                    po = psB.tile([128, 512], F32, tag="B", name=f"pod{n}_{qt}_{ht}")
                    dg_mms(n, qt, ht, po, stop_last=True)
                    nc.scalar.copy(out=dgp[:, qt, ht, :], in_=po)

            # ---------------- phase 2: u  (uT[h, (n,j)], kc-major waves) ---
            for wave in range(2):
                pu = {}
                for kc in range(HC):
                    for ho in range(4):
                        hco = wave * 4 + ho
                        if kc == 0:
                            pu[hco] = psB.tile(
                                [128, B * W], F32, tag="B", name=f"pu{hco}"
                            )
                        nc.tensor.matmul(
                            pu[hco],
                            lhsT=WaT_sb[:, kc, hco * 128:(hco + 1) * 128],
                            rhs=enc_sb[:, kc, :],
                            start=(kc == 0),
                            stop=(kc == HC - 1),
                        )
                for ho in range(4):
                    hco = wave * 4 + ho
                    nc.scalar.copy(out=uT_sb[:, hco, :], in_=pu[hco])

            # ---------------- phase 3: scores + softmax for batches 0-3 ----
            tbs = {}
            for n in range(NB_EARLY):
                score_part(n)
                load_dec(n + NB_EARLY, convert=False)  # reuses freed dec slot
                t = scored.pop(n)
                smx_a(n, t)
                tbs[n] = smx_b(n, t)

            # ---------------- phase 4: v ----------------
            v_group(0)
            v_group(1)

            # ---------------- phase 5: contexts 0-3 (ctx + stored partial) -
            for n in range(NB_EARLY):
                tb = tbs.pop(n)
                dgp = dgp_tiles.pop(n)
                o_sb = out_pool.tile([128, 2, H], F32, tag="o", name=f"o{n}")
                dst = out[n * Q:(n + 1) * Q, :].rearrange("(qt p) h -> p qt h", p=128)
                for qt in range(2):
                    for ht in range(2):
                        po = psB.tile([128, 512], F32, tag="B", name=f"poc{n}_{qt}_{ht}")
                        nc.tensor.matmul(
                            po,
                            lhsT=tb[:, qt * 128:(qt + 1) * 128],
                            rhs=v_sb[:, n, ht * 512:(ht + 1) * 512],
                            start=True,
                            stop=True,
                        )
                        osl = o_sb[:, qt, ht * 512:(ht + 1) * 512]
                        nc.vector.tensor_add(osl, po, dgp[:, qt, ht, :])
                        nc.scalar.activation(out=osl, in_=osl, func=AF.Tanh)
                    eng = nc.sync if qt == 0 else nc.scalar
                    eng.dma_start(out=dst[:, qt, :], in_=o_sb[:, qt, :])

            # ---------------- phase 6: batches 4-7, standard pipeline ------
            prev = None  # (n, pos, o_sb) awaiting tanh + store

            def flush_prev():
                nonlocal prev
                if prev is None:
                    return
                pn, ppos, po_sb = prev
                for qt in range(2):
                    for ht in range(2):
                        nc.scalar.activation(
                            out=po_sb[:, qt, ht * 512:(ht + 1) * 512],
                            in_=ppos[(qt, ht)], func=AF.Tanh,
                        )
                dst = out[pn * Q:(pn + 1) * Q, :].rearrange("(qt p) h -> p qt h", p=128)
                nc.sync.dma_start(out=dst[:, 0, :], in_=po_sb[:, 0, :])
                nc.scalar.dma_start(out=dst[:, 1, :], in_=po_sb[:, 1, :])
                prev = None

            state = {}

            def batch_pre(n):
                convert_dec(n)
                score_part(n)
                t = scored.pop(n)
                flush_prev()
                o_sb = out_pool.tile([128, 2, H], F32, tag="o", name=f"o{n}")
                pos = {}

                def dec_group(qt, ht):
                    po = psB.tile([128, 512], F32, tag="B", name=f"po{n}_{qt}_{ht}")
                    pos[(qt, ht)] = po
                    dg_mms(n, qt, ht, po)

                dec_group(0, 0)
                smx_a(n, t)
                dec_group(1, 0)
                tb = smx_b(n, t)
                dec_group(0, 1)
                dec_group(1, 1)
                state[n] = (tb, pos, o_sb)

            def batch_ctx(n):
                tb, pos, o_sb = state.pop(n)
                last = n == B - 1
                dst = out[n * Q:(n + 1) * Q, :].rearrange("(qt p) h -> p qt h", p=128)
                for qt in range(2):
                    for ht in range(2):
                        nc.tensor.matmul(
                            pos[(qt, ht)],
                            lhsT=tb[:, qt * 128:(qt + 1) * 128],
                            rhs=v_sb[:, n, ht * 512:(ht + 1) * 512],
                            start=False,
                            stop=True,
                        )
                        if last:
                            nc.scalar.activation(
                                out=o_sb[:, qt, ht * 512:(ht + 1) * 512],
                                in_=pos[(qt, ht)], func=AF.Tanh,
                            )
                    if last:
                        eng = nc.sync if qt == 0 else nc.scalar
                        eng.dma_start(out=dst[:, qt, :], in_=o_sb[:, qt, :])
                nonlocal prev
                if not last:
                    prev = (n, pos, o_sb)

            for n in range(NB_EARLY, B):
                batch_pre(n)
                batch_ctx(n)
            flush_prev()
    nc.compile()
    return nc


def round_f32r(a: np.ndarray) -> np.ndarray:
    """Round fp32 to fp32r (TF32-like: 11-bit mantissa, low 12 bits zero),
    round-to-nearest-even.  This is what the PE consumes in fp32r mode."""
    u = np.ascontiguousarray(a, dtype=np.float32).view(np.uint32)
    lsb = (u >> np.uint32(12)) & np.uint32(1)
    u = (u + np.uint32(0x7FF) + lsb) & np.uint32(0xFFFFF000)
    return u.view(np.float32)


def prepare_in_maps(inputs: dict) -> list[dict]:
    enc = np.asarray(inputs["encoder_outputs"], dtype=np.float32)
    dec = np.asarray(inputs["decoder_h_t"], dtype=np.float32)
    src_len = np.asarray(inputs["src_len"], dtype=np.int32)
    p_t = np.asarray(inputs["p_t"], dtype=np.float32)
    W_a = np.asarray(inputs["W_a"], dtype=np.float32)
    W_c = np.asarray(inputs["W_c"], dtype=np.float32)

    # Window bounds, computed with the same fp32 ops as the reference.
    attn_start = np.maximum(p_t - np.float32(WINDOW), np.float32(0.0))
    attn_end = np.minimum(p_t + np.float32(WINDOW), src_len.astype(np.float32))
    idx_full = np.arange(L, dtype=np.float32)
    mask_full = (idx_full[None, :] < attn_start[:, None]) | (
        idx_full[None, :] > attn_end[:, None]
    )
    live = ~mask_full
    s = np.clip(live.argmax(axis=1), 0, L - W)  # first live position per batch
    idx = s[:, None] + np.arange(W)[None, :]
    idxf = idx.astype(np.float32)
    mask = (idxf < attn_start[:, None]) | (idxf > attn_end[:, None])
    bias = np.where(mask, np.float32(MASK_BIAS), np.float32(LOG_ALPHA)).astype(np.float32)
    g = np.exp(-((idxf - p_t[:, None]) ** 2) / np.float32(DEV_POW)).astype(np.float32)

    enc_w = round_f32r(enc[np.arange(N)[:, None], idx, :])  # [N, W, H]
    dec = round_f32r(dec)
    W_aT = round_f32r(W_a.T)
    W_c1Tb = W_c[:, :H].T.astype(ml_dtypes.bfloat16)
    W_c2Tb = W_c[:, H:].T.astype(ml_dtypes.bfloat16)

    in_maps = []
    for c in range(NCORES):
        bs = slice(c * B, (c + 1) * B)
        gc = g[bs]  # [B, W]
        gpack = np.zeros((128, 2), dtype=np.float32)
        for n in range(B):
            gi, off = divmod(n, 4)
            gpack[off * W:(off + 1) * W, gi] = gc[n]
        enc_wT = np.ascontiguousarray(enc_w[bs].transpose(2, 0, 1).reshape(H, B * W))
        in_maps.append({
            "enc_wT": enc_wT,
            "enc_wTb": enc_wT.astype(ml_dtypes.bfloat16),
            "dec_hT": np.ascontiguousarray(dec[bs].transpose(2, 0, 1).reshape(H, B * Q)),
            "W_aT": W_aT,
            "W_c1T": W_c1Tb,
            "W_c2T": W_c2Tb,
            "biasT": np.ascontiguousarray(bias[bs].T),
            "onesD": np.ones((W, W), dtype=np.float32),
            "gPackT": gpack,
        })
    return in_maps


_NC = None


def get_nc() -> bass.Bass:
    global _NC
    if _NC is None:
        _NC = build_nc()
    return _NC


def kernel(**inputs) -> np.ndarray:
    nc = get_nc()
    in_maps = prepare_in_maps(inputs)
    res = run_bass_kernel_spmd(nc, in_maps, list(range(NCORES)))
    outs = [res.results[c]["out"].reshape(B, Q, H) for c in range(NCORES)]
    return np.concatenate(outs, axis=0)
